# revision 3
# baseline (speedup 1.0000x reference)
"""ConstituencyTreeLSTM on 8 Trainium2 NeuronCores (Bass/Tile) — v2.

Data-parallel over the batch of trees: B=128 trees sharded 16/core across 8
cores; all 14 gate weight matrices replicated per core (bf16).

Per-core program (B_local=16 trees, S=1024 leaves), all-bf16 compute with
fp32 PSUM accumulation:
  - x gather: one batched indirect DMA per 512-leaf half from an embedding
    table host-padded to 384 cols; x^T produced by XBAR DMA-transpose on the
    sync-engine HWDGE queue (no PE transposes, no DVE copies).
  - Leaf cell needs only i,o,u gates (child states are zero; f gates
    multiply zero c).
  - All gate GEMMs emitted weight-outer over up to 4 moving 512-col blocks
    so consecutive matmuls share the stationary operand.
  - Phase A processes groups of 2 trees (levels 0..2), software-pipelined:
    round r emits [gather(r+1) | level2(r-2) | level1(r-1) | leaves(r)] so
    the PE stream never waits on activations of the same group.
  - Phase B: levels 3..10 batched across all 16 trees.

Activations/states are feature-on-partition (h^T/c^T: 256 feats -> 2x128
partition chunks, nodes on the free dim) so child gathers are stride-2
slices on the free dim and every GEMM contracts over the partition dim.
"""

import ml_dtypes
import numpy as np

import concourse.bass as bass
import concourse.mybir as mybir
import concourse.tile as tile
from concourse.bass_utils import run_bass_kernel_spmd

F32 = mybir.dt.float32
BF16 = mybir.dt.bfloat16
I32 = mybir.dt.int32
SIG = mybir.ActivationFunctionType.Sigmoid
TANH = mybir.ActivationFunctionType.Tanh

B, S, E, H, V = 128, 1024, 300, 256, 50000
EP = 384  # embedding row padded to 3*128 for the XBAR transpose
N_CORES = 8
B_LOCAL = B // N_CORES
G_TREES = 2                      # trees per phase-A group
N_GROUPS = B_LOCAL // G_TREES    # 8
TRACE = False

# E=300 contraction chunks
KE = [(0, 128), (128, 128), (256, 44)]


def _build(b_local: int) -> bass.Bass:
    nc = bass.Bass()
    G = b_local * S // 128  # token wrap columns (128)

    tok_d = nc.dram_tensor("tok", [128, G], I32, kind="ExternalInput")
    emb_d = nc.dram_tensor("emb", [V, EP], BF16, kind="ExternalInput")
    w5_d = nc.dram_tensor("w5", [E, 768], BF16, kind="ExternalInput")
    ul_d = nc.dram_tensor("ul", [H, 1280], BF16, kind="ExternalInput")
    ur_d = nc.dram_tensor("ur", [H, 1280], BF16, kind="ExternalInput")
    bl_d = nc.dram_tensor("bl", [768], F32, kind="ExternalInput")
    bi_d = nc.dram_tensor("bi", [1280], F32, kind="ExternalInput")
    out_d = nc.dram_tensor("out", [2, 2 * 128, b_local], BF16,
                           kind="ExternalOutput")

    with tile.TileContext(nc) as tc:
        with (
            tc.tile_pool(name="sb", bufs=2) as sp,
            tc.tile_pool(name="pp", bufs=2, space="PSUM") as pp,
        ):
            # --- persistent tiles (weights arrive bf16 from host) ---
            w5sb = sp.tile([128, 3 * 768], BF16, tag="w5", bufs=1)
            for k, (ko, kw) in enumerate(KE):
                nc.sync.dma_start(
                    out=w5sb[:kw, k * 768:(k + 1) * 768], in_=w5_d[ko:ko + kw, :]
                )
            ulsb = sp.tile([128, 2 * 1280], BF16, tag="ul", bufs=1)
            ursb = sp.tile([128, 2 * 1280], BF16, tag="ur", bufs=1)
            for usb, u_d in ((ulsb, ul_d), (ursb, ur_d)):
                for k in range(2):
                    nc.sync.dma_start(
                        out=usb[:, k * 1280:(k + 1) * 1280],
                        in_=u_d[k * 128:(k + 1) * 128, :],
                    )
            blsb = sp.tile([128, 6], F32, tag="bl", bufs=1)
            for m in range(6):
                nc.sync.dma_start(
                    out=blsb[:, m:m + 1], in_=bl_d[m * 128:(m + 1) * 128]
                )
            bisb = sp.tile([128, 10], F32, tag="bi", bufs=1)
            for m in range(10):
                nc.sync.dma_start(
                    out=bisb[:, m:m + 1], in_=bi_d[m * 128:(m + 1) * 128]
                )
            toksb = sp.tile([128, G], I32, tag="tok", bufs=1)
            nc.sync.dma_start(out=toksb[:, :], in_=tok_d[:, :])

            # ============ generic TreeLSTM level (internal nodes) ============
            def level(hin, cin, outh, outc, in_off, out_off, n_out):
                """One internal level: children at cols [in_off, in_off+2*n_out)
                of hin/cin (2 feature-chunk tiles each), n_out output nodes
                written at [out_off, out_off+n_out) of outh/outc.

                Gate m-chunks: i=0,1 f_l=2,3 f_r=4,5 o=6,7 u=8,9.
                Weight-outer over up to 4 moving 512-col blocks.
                """
                blocks = [(blk, min(512, n_out - blk))
                          for blk in range(0, n_out, 512)]
                assert len(blocks) <= 4

                def hl(kk, blk, No):
                    a = in_off + 2 * blk
                    return hin[kk][:, a:a + 2 * No][:, 0::2]

                def hr(kk, blk, No):
                    a = in_off + 2 * blk
                    return hin[kk][:, a:a + 2 * No][:, 1::2]

                gate = {}

                def mm_group(m):
                    ps = {}
                    for blk, No in blocks:
                        ps[blk] = pp.tile([128, No], F32, name="ps", tag="ps",
                                          bufs=8)
                    stats = ((ulsb, 0, hl, True, False),
                             (ulsb, 1, hl, False, False),
                             (ursb, 0, hr, False, False),
                             (ursb, 1, hr, False, True))
                    for wt, kk, mov, st, sto in stats:
                        wap = wt[:, kk * 1280 + m * 128:kk * 1280 + (m + 1) * 128]
                        for blk, No in blocks:
                            nc.tensor.matmul(
                                ps[blk][:, :], wap, mov(kk, blk, No),
                                start=st, stop=sto, skip_group_check=True,
                            )
                    for blk, No in blocks:
                        g = sp.tile([128, No], BF16, name="g", tag="g", bufs=24)
                        nc.scalar.activation(
                            out=g[:, :], in_=ps[blk][:, :],
                            func=(SIG if m < 8 else TANH),
                            bias=bisb[:, m:m + 1],
                        )
                        gate[(m, blk)] = g

                for m in (0, 1, 8, 9):
                    mm_group(m)
                cn = {}
                for blk, No in blocks:
                    for j in range(2):
                        cnj = outc[j][:, out_off + blk:out_off + blk + No]
                        nc.vector.tensor_mul(
                            cnj, gate[(j, blk)][:, :], gate[(8 + j, blk)][:, :]
                        )
                        cn[(j, blk)] = cnj
                for m in (2, 3):
                    mm_group(m)
                for blk, No in blocks:
                    a = in_off + 2 * blk
                    for j in range(2):
                        cl = cin[j][:, a:a + 2 * No][:, 0::2]
                        t2 = sp.tile([128, No], BF16, name="t2", tag="t2",
                                     bufs=12)
                        nc.vector.tensor_mul(t2[:, :], gate[(2 + j, blk)][:, :], cl)
                        nc.vector.tensor_add(cn[(j, blk)], cn[(j, blk)], t2[:, :])
                for m in (4, 5):
                    mm_group(m)
                for blk, No in blocks:
                    a = in_off + 2 * blk
                    for j in range(2):
                        cr = cin[j][:, a:a + 2 * No][:, 1::2]
                        t2 = sp.tile([128, No], BF16, name="t2", tag="t2",
                                     bufs=12)
                        nc.vector.tensor_mul(t2[:, :], gate[(4 + j, blk)][:, :], cr)
                        nc.vector.tensor_add(cn[(j, blk)], cn[(j, blk)], t2[:, :])
                for m in (6, 7):
                    mm_group(m)
                for blk, No in blocks:
                    for j in range(2):
                        tt = sp.tile([128, No], BF16, name="tt", tag="tt",
                                     bufs=8)
                        nc.scalar.activation(
                            out=tt[:, :], in_=cn[(j, blk)], func=TANH)
                        nc.vector.tensor_mul(
                            outh[j][:, out_off + blk:out_off + blk + No],
                            gate[(6 + j, blk)][:, :], tt[:, :],
                        )

            # ============ phase A structures ============
            # L2 outputs, all trees: 16*256 = 4096 cols
            h2 = [sp.tile([128, 4096], BF16, name=f"h2{j}", tag="l2", bufs=4)
                  for j in range(2)]
            c2 = [sp.tile([128, 4096], BF16, name=f"c2{j}", tag="l2", bufs=4)
                  for j in range(2)]

            xT = {}          # half index -> [128, 3, 512] bf16
            l0 = {}          # group -> (h0, c0) tiles [128, 2048]
            l1 = {}          # group -> (h1, c1) tiles [128, 1024]

            def gather(g):
                """Gather + transpose x for group g's 4 halves."""
                for half in range(4):
                    hh = 4 * g + half
                    x = sp.tile([128, 4, EP], BF16, name="x", tag="x", bufs=4)
                    for c4 in range(4):
                        nc.gpsimd.indirect_dma_start(
                            out=x[:, c4, :],
                            out_offset=None,
                            in_=emb_d[:, :],
                            in_offset=bass.IndirectOffsetOnAxis(
                                ap=toksb[:, hh * 4 + c4:hh * 4 + c4 + 1], axis=0
                            ),
                        )
                    xt = sp.tile([128, 3, 512], BF16, name="xT", tag="xT",
                                 bufs=8)
                    for c4 in range(4):
                        nc.sync.dma_start_transpose(
                            out=xt[:, :, c4 * 128:(c4 + 1) * 128],
                            in_=x[:, c4, :],
                        )
                    xT[hh] = x, xt

            def leaves(g):
                """Leaf cell for group g (4 halves x 512 leaves).

                w5 layout i|o|u: gate m-chunks i=0,1 o=2,3 u=4,5."""
                h0 = [sp.tile([128, 2048], BF16, name=f"h0{j}", tag="l0",
                              bufs=8) for j in range(2)]
                c0 = [sp.tile([128, 2048], BF16, name=f"c0{j}", tag="l0",
                              bufs=8) for j in range(2)]
                gate = {}

                def mm_group(m):
                    ps = {}
                    for half in range(4):
                        ps[half] = pp.tile([128, 512], F32, name="ps",
                                           tag="ps", bufs=8)
                    for k, (ko, kw) in enumerate(KE):
                        wap = w5sb[:kw, k * 768 + m * 128:k * 768 + (m + 1) * 128]
                        for half in range(4):
                            nc.tensor.matmul(
                                ps[half][:, :], wap,
                                xT[4 * g + half][1][:kw, k, :],
                                start=(k == 0), stop=(k == 2),
                                skip_group_check=True,
                            )
                    for half in range(4):
                        gm = sp.tile([128, 512], BF16, name="g", tag="g",
                                     bufs=24)
                        nc.scalar.activation(
                            out=gm[:, :], in_=ps[half][:, :],
                            func=(SIG if m < 4 else TANH),
                            bias=blsb[:, m:m + 1],
                        )
                        gate[(m, half)] = gm

                for m in (0, 1, 4, 5):
                    mm_group(m)
                for half in range(4):
                    lo = half * 512
                    for j in range(2):
                        nc.vector.tensor_mul(
                            c0[j][:, lo:lo + 512],
                            gate[(j, half)][:, :], gate[(4 + j, half)][:, :],
                        )
                for m in (2, 3):
                    mm_group(m)
                for half in range(4):
                    lo = half * 512
                    for j in range(2):
                        tt = sp.tile([128, 512], BF16, name="tt", tag="tt",
                                     bufs=8)
                        nc.scalar.activation(
                            out=tt[:, :], in_=c0[j][:, lo:lo + 512], func=TANH)
                        nc.vector.tensor_mul(
                            h0[j][:, lo:lo + 512],
                            gate[(2 + j, half)][:, :], tt[:, :],
                        )
                l0[g] = (h0, c0)

            def level1(g):
                h0, c0 = l0[g]
                h1 = [sp.tile([128, 1024], BF16, name=f"h1{j}", tag="l1",
                              bufs=8) for j in range(2)]
                c1 = [sp.tile([128, 1024], BF16, name=f"c1{j}", tag="l1",
                              bufs=8) for j in range(2)]
                level(h0, c0, h1, c1, 0, 0, 1024)
                l1[g] = (h1, c1)

            def level2(g):
                h1, c1 = l1[g]
                level(h1, c1, h2, c2, 0, g * 512, 512)

            # ---- software-pipelined phase A ----
            gather(0)
            for r in range(N_GROUPS + 2):
                if r + 1 < N_GROUPS:
                    gather(r + 1)
                if r >= 2:
                    level2(r - 2)
                if 1 <= r <= N_GROUPS:
                    level1(r - 1)
                if r < N_GROUPS:
                    leaves(r)

            # ---- phase B: levels 3..10 over all trees ----
            ha, hb = h2
            ca, cb = c2
            n = 4096
            lv = 0
            while n > b_local:
                no_total = n // 2
                tg = "pba" if lv % 2 == 0 else "pbb"
                nh = [sp.tile([128, no_total], BF16, name="nh", tag=tg, bufs=4)
                      for _ in range(2)]
                ncc = [sp.tile([128, no_total], BF16, name="ncc", tag=tg,
                               bufs=4) for _ in range(2)]
                level([ha, hb], [ca, cb], nh, ncc, 0, 0, no_total)
                ha, hb = nh
                ca, cb = ncc
                n = no_total
                lv += 1

            nc.sync.dma_start(out=out_d[0, 0:128, :], in_=ha[:, :])
            nc.sync.dma_start(out=out_d[0, 128:256, :], in_=hb[:, :])
            nc.sync.dma_start(out=out_d[1, 0:128, :], in_=ca[:, :])
            nc.sync.dma_start(out=out_d[1, 128:256, :], in_=cb[:, :])

    nc.finalize()
    _legalize_waits(nc)
    return nc


def _legalize_waits(nc: bass.Bass) -> None:
    """This walrus build encodes at most ONE sync-wait command per
    instruction; Tile's sem assignment emits up to 4. Hoist the extras onto
    same-engine NoOps inserted immediately before the instruction — the
    engine blocks at the NoOp instead, which is the identical blocking
    point in its in-order stream."""
    k = 0
    for fn in nc.m.functions:
        for blk in fn.blocks:
            out = []
            for inst in blk.instructions:
                si = inst.sync_info
                if si is not None and len(si.on_wait) > 1:
                    waits = list(si.on_wait)
                    for w in waits[:-1]:
                        nop = mybir.InstNoOp(name=f"wn{k}", ins=[], outs=[])
                        k += 1
                        nop.engine = inst.engine
                        nop.sync_info = mybir.SyncInfo(on_wait=[w], on_update=[])
                        out.append(nop)
                    inst.sync_info = mybir.SyncInfo(
                        on_wait=[waits[-1]], on_update=list(si.on_update)
                    )
                out.append(inst)
            blk.instructions = out


_CACHE: dict = {}


def _ensure_ntff_hook() -> None:
    """Register the axon NTFF profile hook; the agent image's `antenv`
    lacks `axon_hooks`, so the boot-time registration degraded silently."""
    import sys
    import types

    if "antenv.axon_hooks" in sys.modules:
        return
    mod = types.ModuleType("antenv.axon_hooks")
    state: dict = {}
    mod.set_axon_ntff_profile_hook = lambda h: state.update(h=h)
    mod.get_axon_ntff_profile_hook = lambda: state.get("h")
    sys.modules["antenv.axon_hooks"] = mod
    try:
        import antenv

        antenv.axon_hooks = mod
        from trn_agent_boot.trn_boot import _ntff_profile_via_ctypes

        mod.set_axon_ntff_profile_hook(
            _ntff_profile_via_ctypes("/opt/axon/libaxon_pjrt.so")
        )
    except Exception as e:  # profiling is best-effort
        print(f"ntff hook unavailable: {e}")


def _get_nc() -> bass.Bass:
    key = ("nc", B_LOCAL)
    if key not in _CACHE:
        _CACHE[key] = _build(B_LOCAL)
    return _CACHE[key]


def _host_prep(inputs: dict) -> dict:
    bf = ml_dtypes.bfloat16
    f = lambda name: np.asarray(inputs[name], dtype=np.float32)
    emb = np.zeros((V, EP), dtype=bf)
    emb[:, :E] = f("embedding").astype(bf)
    w5 = np.concatenate([f("w_i"), f("w_o"), f("w_u")], axis=1).astype(bf)
    bl = np.concatenate(
        [
            f("b_wi") + f("b_uil") + f("b_uir"),
            f("b_wo") + f("b_uol") + f("b_uor"),
            f("b_wu") + f("b_uul") + f("b_uur"),
        ]
    )
    ul = np.concatenate(
        [f("u_i_l"), f("u_f_ll"), f("u_f_rr"), f("u_o_l"), f("u_u_l")], axis=1
    ).astype(bf)
    ur = np.concatenate(
        [f("u_i_r"), f("u_f_lr"), f("u_f_rl"), f("u_o_r"), f("u_u_r")], axis=1
    ).astype(bf)
    bi = np.concatenate(
        [
            f("b_wi") + f("b_uil") + f("b_uir"),
            f("b_wf") + f("b_ufll") + f("b_uflr"),
            f("b_wf") + f("b_ufrl") + f("b_ufrr"),
            f("b_wo") + f("b_uol") + f("b_uor"),
            f("b_wu") + f("b_uul") + f("b_uur"),
        ]
    )
    return {
        "emb": np.ascontiguousarray(emb),
        "w5": np.ascontiguousarray(w5),
        "ul": np.ascontiguousarray(ul),
        "ur": np.ascontiguousarray(ur),
        "bl": np.ascontiguousarray(bl),
        "bi": np.ascontiguousarray(bi),
    }


def _wrap_tokens(tok_flat: np.ndarray) -> np.ndarray:
    # wrapped[p, g] = flat[g*128 + p]
    return np.ascontiguousarray(tok_flat.reshape(-1, 128).T.astype(np.int32))


def kernel(**inputs) -> np.ndarray:
    tokens = np.asarray(inputs["tokens"])
    shared = _host_prep(inputs)
    if TRACE:
        _ensure_ntff_hook()
    nc = _get_nc()
    in_maps = []
    for c in range(N_CORES):
        tok = _wrap_tokens(
            tokens[c * B_LOCAL:(c + 1) * B_LOCAL].reshape(-1)
        )
        in_maps.append({"tok": tok, **shared})
    res = run_bass_kernel_spmd(
        nc, in_maps, list(range(N_CORES)), trace=TRACE
    )
    out = np.empty((2, B, H), np.float32)
    for c in range(N_CORES):
        o = np.asarray(res.results[c]["out"]).astype(np.float32)
        out[0, c * B_LOCAL:(c + 1) * B_LOCAL, :] = o[0].T
        out[1, c * B_LOCAL:(c + 1) * B_LOCAL, :] = o[1].T
    if TRACE:
        _CACHE["last_exec_time_ns"] = res.exec_time_ns
    return out


# revision 6
# speedup vs baseline: 1.2876x; 1.2876x over previous
"""ConstituencyTreeLSTM on 8 Trainium2 NeuronCores (Bass/Tile) — v2.

Data-parallel over the batch of trees: B=128 trees sharded 16/core across 8
cores; all 14 gate weight matrices replicated per core (bf16).

Per-core program (B_local=16 trees, S=1024 leaves), all-fp16 compute with
fp32 PSUM accumulation:
  - x gather: one batched indirect DMA per 512-leaf half from an embedding
    table host-padded to 384 cols (fp16); x^T produced by XBAR DMA-transpose on the
    sync-engine HWDGE queue (no PE transposes, no DVE copies).
  - Leaf cell needs only i,o,u gates (child states are zero; f gates
    multiply zero c).
  - All gate GEMMs emitted weight-outer over up to 4 moving 512-col blocks
    so consecutive matmuls share the stationary operand.
  - Phase A processes groups of 2 trees (levels 0..2), software-pipelined:
    round r emits [gather(r+1) | level2(r-2) | level1(r-1) | leaves(r)] so
    the PE stream never waits on activations of the same group.
  - Phase B: levels 3..10 batched across all 16 trees.

Activations/states are feature-on-partition (h^T/c^T: 256 feats -> 2x128
partition chunks, nodes on the free dim) so child gathers are stride-2
slices on the free dim and every GEMM contracts over the partition dim.
"""

import numpy as np

import concourse.bass as bass
import concourse.mybir as mybir
import concourse.tile as tile
from concourse.bass_utils import run_bass_kernel_spmd

F32 = mybir.dt.float32
F16 = mybir.dt.float16
I32 = mybir.dt.int32
SIG = mybir.ActivationFunctionType.Sigmoid
TANH = mybir.ActivationFunctionType.Tanh

B, S, E, H, V = 128, 1024, 300, 256, 50000
EP = 384  # embedding row padded to 3*128 for the XBAR transpose
N_CORES = 8
B_LOCAL = B // N_CORES
G_TREES = 2                      # trees per phase-A group
N_GROUPS = B_LOCAL // G_TREES    # 8
TRACE = False

# E=300 contraction chunks
KE = [(0, 128), (128, 128), (256, 44)]


def _build(b_local: int) -> bass.Bass:
    nc = bass.Bass()
    G = b_local * S // 128  # token wrap columns (128)

    tok_d = nc.dram_tensor("tok", [128, G], I32, kind="ExternalInput")
    emb_d = nc.dram_tensor("emb", [V, EP], F16, kind="ExternalInput")
    w5_d = nc.dram_tensor("w5", [E, 768], F16, kind="ExternalInput")
    ul_d = nc.dram_tensor("ul", [H, 1280], F16, kind="ExternalInput")
    ur_d = nc.dram_tensor("ur", [H, 1280], F16, kind="ExternalInput")
    bl_d = nc.dram_tensor("bl", [768], F32, kind="ExternalInput")
    bi_d = nc.dram_tensor("bi", [1280], F32, kind="ExternalInput")
    out_d = nc.dram_tensor("out", [2, 2 * 128, b_local], F16,
                           kind="ExternalOutput")

    with tile.TileContext(nc) as tc:
        with (
            tc.tile_pool(name="sb", bufs=2) as sp,
            tc.tile_pool(name="pp", bufs=2, space="PSUM") as pp,
        ):
            # --- persistent tiles (weights arrive fp16 from host) ---
            w5sb = sp.tile([128, 3 * 768], F16, tag="w5", bufs=1)
            for k, (ko, kw) in enumerate(KE):
                nc.sync.dma_start(
                    out=w5sb[:kw, k * 768:(k + 1) * 768], in_=w5_d[ko:ko + kw, :]
                )
            ulsb = sp.tile([128, 2 * 1280], F16, tag="ul", bufs=1)
            ursb = sp.tile([128, 2 * 1280], F16, tag="ur", bufs=1)
            for usb, u_d in ((ulsb, ul_d), (ursb, ur_d)):
                for k in range(2):
                    nc.sync.dma_start(
                        out=usb[:, k * 1280:(k + 1) * 1280],
                        in_=u_d[k * 128:(k + 1) * 128, :],
                    )
            blsb = sp.tile([128, 6], F32, tag="bl", bufs=1)
            for m in range(6):
                nc.sync.dma_start(
                    out=blsb[:, m:m + 1], in_=bl_d[m * 128:(m + 1) * 128]
                )
            bisb = sp.tile([128, 10], F32, tag="bi", bufs=1)
            for m in range(10):
                nc.sync.dma_start(
                    out=bisb[:, m:m + 1], in_=bi_d[m * 128:(m + 1) * 128]
                )
            toksb = sp.tile([128, G], I32, tag="tok", bufs=1)
            nc.sync.dma_start(out=toksb[:, :], in_=tok_d[:, :])

            # ============ generic TreeLSTM level (internal nodes) ============
            def level(hin, cin, outh, outc, in_off, out_off, n_out):
                """One internal level: children at cols [in_off, in_off+2*n_out)
                of hin/cin (2 feature-chunk tiles each), n_out output nodes
                written at [out_off, out_off+n_out) of outh/outc.

                Gate m-chunks: i=0,1 f_l=2,3 f_r=4,5 o=6,7 u=8,9.
                Weight-outer over up to 4 moving 512-col blocks.
                """
                blocks = [(blk, min(512, n_out - blk))
                          for blk in range(0, n_out, 512)]
                assert len(blocks) <= 4

                def hl(kk, blk, No):
                    a = in_off + 2 * blk
                    return hin[kk][:, a:a + 2 * No][:, 0::2]

                def hr(kk, blk, No):
                    a = in_off + 2 * blk
                    return hin[kk][:, a:a + 2 * No][:, 1::2]

                gate = {}

                def mm_group(m):
                    ps = {}
                    for blk, No in blocks:
                        ps[blk] = pp.tile([128, No], F32, name="ps", tag="ps",
                                          bufs=8)
                    stats = ((ulsb, 0, hl, True, False),
                             (ulsb, 1, hl, False, False),
                             (ursb, 0, hr, False, False),
                             (ursb, 1, hr, False, True))
                    for wt, kk, mov, st, sto in stats:
                        wap = wt[:, kk * 1280 + m * 128:kk * 1280 + (m + 1) * 128]
                        for blk, No in blocks:
                            nc.tensor.matmul(
                                ps[blk][:, :], wap, mov(kk, blk, No),
                                start=st, stop=sto, skip_group_check=True,
                            )
                    for blk, No in blocks:
                        g = sp.tile([128, No], F16, name="g", tag="g", bufs=22)
                        nc.scalar.activation(
                            out=g[:, :], in_=ps[blk][:, :],
                            func=(SIG if m < 8 else TANH),
                            bias=bisb[:, m:m + 1],
                        )
                        gate[(m, blk)] = g

                for m in (0, 1, 8, 9):
                    mm_group(m)
                cn = {}
                for blk, No in blocks:
                    for j in range(2):
                        cnj = outc[j][:, out_off + blk:out_off + blk + No]
                        nc.vector.tensor_mul(
                            cnj, gate[(j, blk)][:, :], gate[(8 + j, blk)][:, :]
                        )
                        cn[(j, blk)] = cnj
                for m in (2, 3):
                    mm_group(m)
                for blk, No in blocks:
                    a = in_off + 2 * blk
                    for j in range(2):
                        cl = cin[j][:, a:a + 2 * No][:, 0::2]
                        t2 = sp.tile([128, No], F16, name="t2", tag="t2",
                                     bufs=10)
                        nc.vector.tensor_mul(t2[:, :], gate[(2 + j, blk)][:, :], cl)
                        nc.vector.tensor_add(cn[(j, blk)], cn[(j, blk)], t2[:, :])
                for m in (4, 5):
                    mm_group(m)
                for blk, No in blocks:
                    a = in_off + 2 * blk
                    for j in range(2):
                        cr = cin[j][:, a:a + 2 * No][:, 1::2]
                        t2 = sp.tile([128, No], F16, name="t2", tag="t2",
                                     bufs=10)
                        nc.vector.tensor_mul(t2[:, :], gate[(4 + j, blk)][:, :], cr)
                        nc.vector.tensor_add(cn[(j, blk)], cn[(j, blk)], t2[:, :])
                for m in (6, 7):
                    mm_group(m)
                for blk, No in blocks:
                    for j in range(2):
                        tt = sp.tile([128, No], F16, name="tt", tag="tt",
                                     bufs=8)
                        nc.scalar.activation(
                            out=tt[:, :], in_=cn[(j, blk)], func=TANH)
                        nc.vector.tensor_mul(
                            outh[j][:, out_off + blk:out_off + blk + No],
                            gate[(6 + j, blk)][:, :], tt[:, :],
                        )

            # ============ phase A structures ============
            # L2 outputs, all trees: 16*256 = 4096 cols
            h2 = [sp.tile([128, 4096], F16, name=f"h2{j}", tag="l2", bufs=4)
                  for j in range(2)]
            c2 = [sp.tile([128, 4096], F16, name=f"c2{j}", tag="l2", bufs=4)
                  for j in range(2)]

            xT = {}          # half index -> [128, 3, 512] bf16
            l0 = {}          # group -> (h0, c0) tiles [128, 2048]
            l1 = {}          # group -> (h1, c1) tiles [128, 1024]

            def gather(g):
                """Gather + transpose x for group g's 4 halves."""
                for half in range(4):
                    hh = 4 * g + half
                    x = sp.tile([128, 4, EP], F16, name="x", tag="x", bufs=6)
                    for c4 in range(4):
                        nc.gpsimd.indirect_dma_start(
                            out=x[:, c4, :],
                            out_offset=None,
                            in_=emb_d[:, :],
                            in_offset=bass.IndirectOffsetOnAxis(
                                ap=toksb[:, hh * 4 + c4:hh * 4 + c4 + 1], axis=0
                            ),
                        )
                    # c4-major transposed layout: xt[p, c4, k, q] = x[q, c4, k*128+p]
                    xt = sp.tile([128, 4, 3, 128], F16, name="xT", tag="xT",
                                 bufs=8)
                    nc.sync.dma_start_transpose(out=xt[:, :, :, :],
                                                in_=x[:, :, :])
                    xT[hh] = x, xt

            def leaves(g):
                """Leaf cell for group g (4 halves x 512 leaves).

                w5 layout i|o|u: gate m-chunks i=0,1 o=2,3 u=4,5."""
                h0 = [sp.tile([128, 2048], F16, name=f"h0{j}", tag="l0",
                              bufs=8) for j in range(2)]
                c0 = [sp.tile([128, 2048], F16, name=f"c0{j}", tag="l0",
                              bufs=8) for j in range(2)]
                gate = {}

                def mm_group(m):
                    ps = {}
                    for half in range(4):
                        ps[half] = pp.tile([128, 512], F32, name="ps",
                                           tag="ps", bufs=8)
                    for k, (ko, kw) in enumerate(KE):
                        wap = w5sb[:kw, k * 768 + m * 128:k * 768 + (m + 1) * 128]
                        for half in range(4):
                            nc.tensor.matmul(
                                ps[half][:, :], wap,
                                xT[4 * g + half][1][:kw, :, k, :],
                                start=(k == 0), stop=(k == 2),
                                skip_group_check=True,
                            )
                    for half in range(4):
                        gm = sp.tile([128, 512], F16, name="g", tag="g",
                                     bufs=22)
                        nc.scalar.activation(
                            out=gm[:, :], in_=ps[half][:, :],
                            func=(SIG if m < 4 else TANH),
                            bias=blsb[:, m:m + 1],
                        )
                        gate[(m, half)] = gm

                for m in (0, 1, 4, 5):
                    mm_group(m)
                for half in range(4):
                    lo = half * 512
                    for j in range(2):
                        nc.vector.tensor_mul(
                            c0[j][:, lo:lo + 512],
                            gate[(j, half)][:, :], gate[(4 + j, half)][:, :],
                        )
                for m in (2, 3):
                    mm_group(m)
                for half in range(4):
                    lo = half * 512
                    for j in range(2):
                        tt = sp.tile([128, 512], F16, name="tt", tag="tt",
                                     bufs=8)
                        nc.scalar.activation(
                            out=tt[:, :], in_=c0[j][:, lo:lo + 512], func=TANH)
                        nc.vector.tensor_mul(
                            h0[j][:, lo:lo + 512],
                            gate[(2 + j, half)][:, :], tt[:, :],
                        )
                l0[g] = (h0, c0)

            def level1(g):
                h0, c0 = l0[g]
                h1 = [sp.tile([128, 1024], F16, name=f"h1{j}", tag="l1",
                              bufs=8) for j in range(2)]
                c1 = [sp.tile([128, 1024], F16, name=f"c1{j}", tag="l1",
                              bufs=8) for j in range(2)]
                level(h0, c0, h1, c1, 0, 0, 1024)
                l1[g] = (h1, c1)

            def level2(g):
                h1, c1 = l1[g]
                level(h1, c1, h2, c2, 0, g * 512, 512)

            # ---- software-pipelined phase A ----
            gather(0)
            gather(1)
            for r in range(N_GROUPS + 2):
                if r + 2 < N_GROUPS:
                    gather(r + 2)
                if r >= 2:
                    level2(r - 2)
                if 1 <= r <= N_GROUPS:
                    level1(r - 1)
                if r < N_GROUPS:
                    leaves(r)

            # ---- phase B: levels 3..10 over all trees ----
            ha, hb = h2
            ca, cb = c2
            n = 4096
            lv = 0
            while n > b_local:
                no_total = n // 2
                tg = "pba" if lv % 2 == 0 else "pbb"
                nh = [sp.tile([128, no_total], F16, name="nh", tag=tg, bufs=4)
                      for _ in range(2)]
                ncc = [sp.tile([128, no_total], F16, name="ncc", tag=tg,
                               bufs=4) for _ in range(2)]
                level([ha, hb], [ca, cb], nh, ncc, 0, 0, no_total)
                ha, hb = nh
                ca, cb = ncc
                n = no_total
                lv += 1

            nc.sync.dma_start(out=out_d[0, 0:128, :], in_=ha[:, :])
            nc.sync.dma_start(out=out_d[0, 128:256, :], in_=hb[:, :])
            nc.sync.dma_start(out=out_d[1, 0:128, :], in_=ca[:, :])
            nc.sync.dma_start(out=out_d[1, 128:256, :], in_=cb[:, :])

    nc.finalize()
    _legalize_waits(nc)
    return nc


def _legalize_waits(nc: bass.Bass) -> None:
    """This walrus build encodes at most ONE sync-wait command per
    instruction; Tile's sem assignment emits up to 4. Hoist the extras onto
    same-engine NoOps inserted immediately before the instruction — the
    engine blocks at the NoOp instead, which is the identical blocking
    point in its in-order stream."""
    k = 0
    for fn in nc.m.functions:
        for blk in fn.blocks:
            out = []
            for inst in blk.instructions:
                si = inst.sync_info
                if si is not None and len(si.on_wait) > 1:
                    waits = list(si.on_wait)
                    for w in waits[:-1]:
                        nop = mybir.InstNoOp(name=f"wn{k}", ins=[], outs=[])
                        k += 1
                        nop.engine = inst.engine
                        nop.sync_info = mybir.SyncInfo(on_wait=[w], on_update=[])
                        out.append(nop)
                    inst.sync_info = mybir.SyncInfo(
                        on_wait=[waits[-1]], on_update=list(si.on_update)
                    )
                out.append(inst)
            blk.instructions = out


_CACHE: dict = {}


def _ensure_ntff_hook() -> None:
    """Register the axon NTFF profile hook; the agent image's `antenv`
    lacks `axon_hooks`, so the boot-time registration degraded silently."""
    import sys
    import types

    if "antenv.axon_hooks" in sys.modules:
        return
    mod = types.ModuleType("antenv.axon_hooks")
    state: dict = {}
    mod.set_axon_ntff_profile_hook = lambda h: state.update(h=h)
    mod.get_axon_ntff_profile_hook = lambda: state.get("h")
    sys.modules["antenv.axon_hooks"] = mod
    try:
        import antenv

        antenv.axon_hooks = mod
        from trn_agent_boot.trn_boot import _ntff_profile_via_ctypes

        mod.set_axon_ntff_profile_hook(
            _ntff_profile_via_ctypes("/opt/axon/libaxon_pjrt.so")
        )
    except Exception as e:  # profiling is best-effort
        print(f"ntff hook unavailable: {e}")


def _get_nc() -> bass.Bass:
    key = ("nc", B_LOCAL)
    if key not in _CACHE:
        _CACHE[key] = _build(B_LOCAL)
    return _CACHE[key]


def _host_prep(inputs: dict) -> dict:
    bf = np.float16
    f = lambda name: np.asarray(inputs[name], dtype=np.float32)
    emb = np.zeros((V, EP), dtype=bf)
    emb[:, :E] = f("embedding").astype(bf)
    w5 = np.concatenate([f("w_i"), f("w_o"), f("w_u")], axis=1).astype(bf)
    bl = np.concatenate(
        [
            f("b_wi") + f("b_uil") + f("b_uir"),
            f("b_wo") + f("b_uol") + f("b_uor"),
            f("b_wu") + f("b_uul") + f("b_uur"),
        ]
    )
    ul = np.concatenate(
        [f("u_i_l"), f("u_f_ll"), f("u_f_rr"), f("u_o_l"), f("u_u_l")], axis=1
    ).astype(bf)
    ur = np.concatenate(
        [f("u_i_r"), f("u_f_lr"), f("u_f_rl"), f("u_o_r"), f("u_u_r")], axis=1
    ).astype(bf)
    bi = np.concatenate(
        [
            f("b_wi") + f("b_uil") + f("b_uir"),
            f("b_wf") + f("b_ufll") + f("b_uflr"),
            f("b_wf") + f("b_ufrl") + f("b_ufrr"),
            f("b_wo") + f("b_uol") + f("b_uor"),
            f("b_wu") + f("b_uul") + f("b_uur"),
        ]
    )
    return {
        "emb": np.ascontiguousarray(emb),
        "w5": np.ascontiguousarray(w5),
        "ul": np.ascontiguousarray(ul),
        "ur": np.ascontiguousarray(ur),
        "bl": np.ascontiguousarray(bl),
        "bi": np.ascontiguousarray(bi),
    }


def _wrap_tokens(tok_flat: np.ndarray) -> np.ndarray:
    # wrapped[p, g] = flat[g*128 + p]
    return np.ascontiguousarray(tok_flat.reshape(-1, 128).T.astype(np.int32))


def kernel(**inputs) -> np.ndarray:
    tokens = np.asarray(inputs["tokens"])
    shared = _host_prep(inputs)
    if TRACE:
        _ensure_ntff_hook()
    nc = _get_nc()
    in_maps = []
    for c in range(N_CORES):
        tok = _wrap_tokens(
            tokens[c * B_LOCAL:(c + 1) * B_LOCAL].reshape(-1)
        )
        in_maps.append({"tok": tok, **shared})
    res = run_bass_kernel_spmd(
        nc, in_maps, list(range(N_CORES)), trace=TRACE
    )
    out = np.empty((2, B, H), np.float32)
    for c in range(N_CORES):
        o = np.asarray(res.results[c]["out"]).astype(np.float32)
        out[0, c * B_LOCAL:(c + 1) * B_LOCAL, :] = o[0].T
        out[1, c * B_LOCAL:(c + 1) * B_LOCAL, :] = o[1].T
    if TRACE:
        _CACHE["last_exec_time_ns"] = res.exec_time_ns
    return out


# revision 7
# speedup vs baseline: 1.3102x; 1.0176x over previous
"""ConstituencyTreeLSTM on 8 Trainium2 NeuronCores (Bass/Tile) — v2.

Data-parallel over the batch of trees: B=128 trees sharded 16/core across 8
cores; all 14 gate weight matrices replicated per core (bf16).

Per-core program (B_local=16 trees, S=1024 leaves), all-fp16 compute with
fp32 PSUM accumulation:
  - x gather: one batched indirect DMA per 512-leaf half from an embedding
    table host-padded to 384 cols (fp16); x^T produced by XBAR DMA-transpose on the
    sync-engine HWDGE queue (no PE transposes, no DVE copies).
  - Leaf cell needs only i,o,u gates (child states are zero; f gates
    multiply zero c).
  - All gate GEMMs emitted weight-outer over up to 4 moving 512-col blocks
    so consecutive matmuls share the stationary operand.
  - Phase A processes groups of 2 trees (levels 0..2), software-pipelined:
    round r emits [gather(r+1) | level2(r-2) | level1(r-1) | leaves(r)] so
    the PE stream never waits on activations of the same group.
  - Phase B: levels 3..10 batched across all 16 trees.

Activations/states are feature-on-partition (h^T/c^T: 256 feats -> 2x128
partition chunks, nodes on the free dim) so child gathers are stride-2
slices on the free dim and every GEMM contracts over the partition dim.
"""

import numpy as np

import concourse.bass as bass
import concourse.mybir as mybir
import concourse.tile as tile
from concourse.bass_utils import run_bass_kernel_spmd

F32 = mybir.dt.float32
F16 = mybir.dt.float16
I32 = mybir.dt.int32
SIG = mybir.ActivationFunctionType.Sigmoid
TANH = mybir.ActivationFunctionType.Tanh

B, S, E, H, V = 128, 1024, 300, 256, 50000
EP = 384  # embedding row padded to 3*128 for the XBAR transpose
N_CORES = 8
B_LOCAL = B // N_CORES
G_TREES = 2                      # trees per phase-A group
N_GROUPS = B_LOCAL // G_TREES    # 8
TRACE = False

# E=300 contraction chunks
KE = [(0, 128), (128, 128), (256, 44)]


def _build(b_local: int) -> bass.Bass:
    nc = bass.Bass()
    G = b_local * S // 128  # token wrap columns (128)

    tok_d = nc.dram_tensor("tok", [128, G], I32, kind="ExternalInput")
    emb_d = nc.dram_tensor("emb", [V, EP], F16, kind="ExternalInput")
    w5_d = nc.dram_tensor("w5", [E, 768], F16, kind="ExternalInput")
    ul_d = nc.dram_tensor("ul", [H, 1280], F16, kind="ExternalInput")
    ur_d = nc.dram_tensor("ur", [H, 1280], F16, kind="ExternalInput")
    bl_d = nc.dram_tensor("bl", [768], F32, kind="ExternalInput")
    bi_d = nc.dram_tensor("bi", [1280], F32, kind="ExternalInput")
    out_d = nc.dram_tensor("out", [2, 2 * 128, b_local], F16,
                           kind="ExternalOutput")

    with tile.TileContext(nc) as tc:
        with (
            tc.tile_pool(name="sb", bufs=2) as sp,
            tc.tile_pool(name="pp", bufs=2, space="PSUM") as pp,
        ):
            # --- persistent tiles (weights arrive fp16 from host) ---
            toksb = sp.tile([128, G], I32, tag="tok", bufs=1)
            nc.sync.dma_start(out=toksb[:, :], in_=tok_d[:, :])
            w5sb = sp.tile([128, 3 * 768], F16, tag="w5", bufs=1)
            for k, (ko, kw) in enumerate(KE):
                nc.sync.dma_start(
                    out=w5sb[:kw, k * 768:(k + 1) * 768], in_=w5_d[ko:ko + kw, :]
                )
            ulsb = sp.tile([128, 2 * 1280], F16, tag="ul", bufs=1)
            ursb = sp.tile([128, 2 * 1280], F16, tag="ur", bufs=1)
            for usb, u_d in ((ulsb, ul_d), (ursb, ur_d)):
                for k in range(2):
                    nc.sync.dma_start(
                        out=usb[:, k * 1280:(k + 1) * 1280],
                        in_=u_d[k * 128:(k + 1) * 128, :],
                    )
            blsb = sp.tile([128, 6], F32, tag="bl", bufs=1)
            for m in range(6):
                nc.sync.dma_start(
                    out=blsb[:, m:m + 1], in_=bl_d[m * 128:(m + 1) * 128]
                )
            bisb = sp.tile([128, 10], F32, tag="bi", bufs=1)
            for m in range(10):
                nc.sync.dma_start(
                    out=bisb[:, m:m + 1], in_=bi_d[m * 128:(m + 1) * 128]
                )

            # ============ generic TreeLSTM level (internal nodes) ============
            def level(hin, cin, outh, outc, in_off, out_off, n_out):
                """One internal level: children at cols [in_off, in_off+2*n_out)
                of hin/cin (2 feature-chunk tiles each), n_out output nodes
                written at [out_off, out_off+n_out) of outh/outc.

                Gate m-chunks: i=0,1 f_l=2,3 f_r=4,5 o=6,7 u=8,9.
                Weight-outer over up to 4 moving 512-col blocks.
                """
                blocks = [(blk, min(512, n_out - blk))
                          for blk in range(0, n_out, 512)]
                assert len(blocks) <= 4

                def hl(kk, blk, No):
                    a = in_off + 2 * blk
                    return hin[kk][:, a:a + 2 * No][:, 0::2]

                def hr(kk, blk, No):
                    a = in_off + 2 * blk
                    return hin[kk][:, a:a + 2 * No][:, 1::2]

                gate = {}

                def mm_group(m):
                    ps = {}
                    for blk, No in blocks:
                        ps[blk] = pp.tile([128, No], F32, name="ps", tag="ps",
                                          bufs=8)
                    stats = ((ulsb, 0, hl, True, False),
                             (ulsb, 1, hl, False, False),
                             (ursb, 0, hr, False, False),
                             (ursb, 1, hr, False, True))
                    for wt, kk, mov, st, sto in stats:
                        wap = wt[:, kk * 1280 + m * 128:kk * 1280 + (m + 1) * 128]
                        for blk, No in blocks:
                            nc.tensor.matmul(
                                ps[blk][:, :], wap, mov(kk, blk, No),
                                start=st, stop=sto, skip_group_check=True,
                            )
                    for blk, No in blocks:
                        g = sp.tile([128, No], F16, name="g", tag="g", bufs=22)
                        nc.scalar.activation(
                            out=g[:, :], in_=ps[blk][:, :],
                            func=(SIG if m < 8 else TANH),
                            bias=bisb[:, m:m + 1],
                        )
                        gate[(m, blk)] = g

                for m in (0, 1, 8, 9):
                    mm_group(m)
                cn = {}
                for blk, No in blocks:
                    for j in range(2):
                        cnj = outc[j][:, out_off + blk:out_off + blk + No]
                        nc.vector.tensor_mul(
                            cnj, gate[(j, blk)][:, :], gate[(8 + j, blk)][:, :]
                        )
                        cn[(j, blk)] = cnj
                for m in (2, 3):
                    mm_group(m)
                for blk, No in blocks:
                    a = in_off + 2 * blk
                    for j in range(2):
                        cl = cin[j][:, a:a + 2 * No][:, 0::2]
                        t2 = sp.tile([128, No], F16, name="t2", tag="t2",
                                     bufs=10)
                        nc.vector.tensor_mul(t2[:, :], gate[(2 + j, blk)][:, :], cl)
                        nc.vector.tensor_add(cn[(j, blk)], cn[(j, blk)], t2[:, :])
                for m in (4, 5):
                    mm_group(m)
                for blk, No in blocks:
                    a = in_off + 2 * blk
                    for j in range(2):
                        cr = cin[j][:, a:a + 2 * No][:, 1::2]
                        t2 = sp.tile([128, No], F16, name="t2", tag="t2",
                                     bufs=10)
                        nc.vector.tensor_mul(t2[:, :], gate[(4 + j, blk)][:, :], cr)
                        nc.vector.tensor_add(cn[(j, blk)], cn[(j, blk)], t2[:, :])
                for m in (6, 7):
                    mm_group(m)
                for blk, No in blocks:
                    for j in range(2):
                        tt = sp.tile([128, No], F16, name="tt", tag="tt",
                                     bufs=8)
                        nc.scalar.activation(
                            out=tt[:, :], in_=cn[(j, blk)], func=TANH)
                        nc.vector.tensor_mul(
                            outh[j][:, out_off + blk:out_off + blk + No],
                            gate[(6 + j, blk)][:, :], tt[:, :],
                        )

            # ============ phase A structures ============
            # L2 outputs, all trees: 16*256 = 4096 cols
            h2 = [sp.tile([128, 4096], F16, name=f"h2{j}", tag="l2", bufs=4)
                  for j in range(2)]
            c2 = [sp.tile([128, 4096], F16, name=f"c2{j}", tag="l2", bufs=4)
                  for j in range(2)]

            xT = {}          # half index -> [128, 3, 512] bf16
            l0 = {}          # group -> (h0, c0) tiles [128, 2048]
            l1 = {}          # group -> (h1, c1) tiles [128, 1024]

            def gather(g):
                """Gather + transpose x for group g's 4 halves."""
                for half in range(4):
                    hh = 4 * g + half
                    x = sp.tile([128, 4, EP], F16, name="x", tag="x", bufs=6)
                    for c4 in range(4):
                        nc.gpsimd.indirect_dma_start(
                            out=x[:, c4, :],
                            out_offset=None,
                            in_=emb_d[:, :],
                            in_offset=bass.IndirectOffsetOnAxis(
                                ap=toksb[:, hh * 4 + c4:hh * 4 + c4 + 1], axis=0
                            ),
                        )
                    # c4-major transposed layout: xt[p, c4, k, q] = x[q, c4, k*128+p]
                    xt = sp.tile([128, 4, 3, 128], F16, name="xT", tag="xT",
                                 bufs=8)
                    nc.sync.dma_start_transpose(out=xt[:, :, :, :],
                                                in_=x[:, :, :])
                    xT[hh] = x, xt

            def leaves(g):
                """Leaf cell for group g (4 halves x 512 leaves).

                w5 layout i|o|u: gate m-chunks i=0,1 o=2,3 u=4,5."""
                h0 = [sp.tile([128, 2048], F16, name=f"h0{j}", tag="l0",
                              bufs=8) for j in range(2)]
                c0 = [sp.tile([128, 2048], F16, name=f"c0{j}", tag="l0",
                              bufs=8) for j in range(2)]
                gate = {}

                def mm_group(m):
                    ps = {}
                    for half in range(4):
                        ps[half] = pp.tile([128, 512], F32, name="ps",
                                           tag="ps", bufs=8)
                    for k, (ko, kw) in enumerate(KE):
                        wap = w5sb[:kw, k * 768 + m * 128:k * 768 + (m + 1) * 128]
                        for half in range(4):
                            nc.tensor.matmul(
                                ps[half][:, :], wap,
                                xT[4 * g + half][1][:kw, :, k, :],
                                start=(k == 0), stop=(k == 2),
                                skip_group_check=True,
                            )
                    for half in range(4):
                        gm = sp.tile([128, 512], F16, name="g", tag="g",
                                     bufs=22)
                        nc.scalar.activation(
                            out=gm[:, :], in_=ps[half][:, :],
                            func=(SIG if m < 4 else TANH),
                            bias=blsb[:, m:m + 1],
                        )
                        gate[(m, half)] = gm

                for m in (0, 1, 4, 5):
                    mm_group(m)
                for half in range(4):
                    lo = half * 512
                    for j in range(2):
                        nc.vector.tensor_mul(
                            c0[j][:, lo:lo + 512],
                            gate[(j, half)][:, :], gate[(4 + j, half)][:, :],
                        )
                for m in (2, 3):
                    mm_group(m)
                for half in range(4):
                    lo = half * 512
                    for j in range(2):
                        tt = sp.tile([128, 512], F16, name="tt", tag="tt",
                                     bufs=8)
                        nc.scalar.activation(
                            out=tt[:, :], in_=c0[j][:, lo:lo + 512], func=TANH)
                        nc.vector.tensor_mul(
                            h0[j][:, lo:lo + 512],
                            gate[(2 + j, half)][:, :], tt[:, :],
                        )
                l0[g] = (h0, c0)

            def level1(g):
                h0, c0 = l0[g]
                h1 = [sp.tile([128, 1024], F16, name=f"h1{j}", tag="l1",
                              bufs=8) for j in range(2)]
                c1 = [sp.tile([128, 1024], F16, name=f"c1{j}", tag="l1",
                              bufs=8) for j in range(2)]
                level(h0, c0, h1, c1, 0, 0, 1024)
                l1[g] = (h1, c1)

            def level2(g):
                h1, c1 = l1[g]
                level(h1, c1, h2, c2, 0, g * 512, 512)

            # ---- software-pipelined phase A ----
            gather(0)
            gather(1)
            gather(2)
            for r in range(N_GROUPS + 3):
                if r + 3 < N_GROUPS:
                    gather(r + 3)
                if 3 <= r:
                    level2(r - 3)
                if 2 <= r <= N_GROUPS + 1:
                    level1(r - 2)
                if r < N_GROUPS:
                    leaves(r)

            # ---- phase B: levels 3..10, two independent 8-tree chains ----
            chains = [{"h": h2, "c": c2, "off": 2048 * ch} for ch in range(2)]
            n = 2048
            lv = 0
            while n > b_local // 2:
                no_total = n // 2
                tg = "pba" if lv % 2 == 0 else "pbb"
                for ch in chains:
                    nh = [sp.tile([128, no_total], F16, name="nh", tag=tg,
                                  bufs=8) for _ in range(2)]
                    ncc = [sp.tile([128, no_total], F16, name="ncc", tag=tg,
                                   bufs=8) for _ in range(2)]
                    level(ch["h"], ch["c"], nh, ncc, ch["off"], 0, no_total)
                    ch["h"], ch["c"], ch["off"] = nh, ncc, 0
                n = no_total
                lv += 1

            for ci, ch in enumerate(chains):
                cs = slice(ci * 8, ci * 8 + 8)
                nc.sync.dma_start(out=out_d[0, 0:128, cs], in_=ch["h"][0][:, :])
                nc.sync.dma_start(out=out_d[0, 128:256, cs], in_=ch["h"][1][:, :])
                nc.sync.dma_start(out=out_d[1, 0:128, cs], in_=ch["c"][0][:, :])
                nc.sync.dma_start(out=out_d[1, 128:256, cs], in_=ch["c"][1][:, :])

    nc.finalize()
    _legalize_waits(nc)
    return nc


def _legalize_waits(nc: bass.Bass) -> None:
    """This walrus build encodes at most ONE sync-wait command per
    instruction; Tile's sem assignment emits up to 4. Hoist the extras onto
    same-engine NoOps inserted immediately before the instruction — the
    engine blocks at the NoOp instead, which is the identical blocking
    point in its in-order stream."""
    k = 0
    for fn in nc.m.functions:
        for blk in fn.blocks:
            out = []
            for inst in blk.instructions:
                si = inst.sync_info
                if si is not None and len(si.on_wait) > 1:
                    waits = list(si.on_wait)
                    for w in waits[:-1]:
                        nop = mybir.InstNoOp(name=f"wn{k}", ins=[], outs=[])
                        k += 1
                        nop.engine = inst.engine
                        nop.sync_info = mybir.SyncInfo(on_wait=[w], on_update=[])
                        out.append(nop)
                    inst.sync_info = mybir.SyncInfo(
                        on_wait=[waits[-1]], on_update=list(si.on_update)
                    )
                out.append(inst)
            blk.instructions = out


_CACHE: dict = {}


def _ensure_ntff_hook() -> None:
    """Register the axon NTFF profile hook; the agent image's `antenv`
    lacks `axon_hooks`, so the boot-time registration degraded silently."""
    import sys
    import types

    if "antenv.axon_hooks" in sys.modules:
        return
    mod = types.ModuleType("antenv.axon_hooks")
    state: dict = {}
    mod.set_axon_ntff_profile_hook = lambda h: state.update(h=h)
    mod.get_axon_ntff_profile_hook = lambda: state.get("h")
    sys.modules["antenv.axon_hooks"] = mod
    try:
        import antenv

        antenv.axon_hooks = mod
        from trn_agent_boot.trn_boot import _ntff_profile_via_ctypes

        mod.set_axon_ntff_profile_hook(
            _ntff_profile_via_ctypes("/opt/axon/libaxon_pjrt.so")
        )
    except Exception as e:  # profiling is best-effort
        print(f"ntff hook unavailable: {e}")


def _get_nc() -> bass.Bass:
    key = ("nc", B_LOCAL)
    if key not in _CACHE:
        _CACHE[key] = _build(B_LOCAL)
    return _CACHE[key]


def _host_prep(inputs: dict) -> dict:
    bf = np.float16
    f = lambda name: np.asarray(inputs[name], dtype=np.float32)
    emb = np.zeros((V, EP), dtype=bf)
    emb[:, :E] = f("embedding").astype(bf)
    w5 = np.concatenate([f("w_i"), f("w_o"), f("w_u")], axis=1).astype(bf)
    bl = np.concatenate(
        [
            f("b_wi") + f("b_uil") + f("b_uir"),
            f("b_wo") + f("b_uol") + f("b_uor"),
            f("b_wu") + f("b_uul") + f("b_uur"),
        ]
    )
    ul = np.concatenate(
        [f("u_i_l"), f("u_f_ll"), f("u_f_rr"), f("u_o_l"), f("u_u_l")], axis=1
    ).astype(bf)
    ur = np.concatenate(
        [f("u_i_r"), f("u_f_lr"), f("u_f_rl"), f("u_o_r"), f("u_u_r")], axis=1
    ).astype(bf)
    bi = np.concatenate(
        [
            f("b_wi") + f("b_uil") + f("b_uir"),
            f("b_wf") + f("b_ufll") + f("b_uflr"),
            f("b_wf") + f("b_ufrl") + f("b_ufrr"),
            f("b_wo") + f("b_uol") + f("b_uor"),
            f("b_wu") + f("b_uul") + f("b_uur"),
        ]
    )
    return {
        "emb": np.ascontiguousarray(emb),
        "w5": np.ascontiguousarray(w5),
        "ul": np.ascontiguousarray(ul),
        "ur": np.ascontiguousarray(ur),
        "bl": np.ascontiguousarray(bl),
        "bi": np.ascontiguousarray(bi),
    }


def _wrap_tokens(tok_flat: np.ndarray) -> np.ndarray:
    # wrapped[p, g] = flat[g*128 + p]
    return np.ascontiguousarray(tok_flat.reshape(-1, 128).T.astype(np.int32))


def kernel(**inputs) -> np.ndarray:
    tokens = np.asarray(inputs["tokens"])
    shared = _host_prep(inputs)
    if TRACE:
        _ensure_ntff_hook()
    nc = _get_nc()
    in_maps = []
    for c in range(N_CORES):
        tok = _wrap_tokens(
            tokens[c * B_LOCAL:(c + 1) * B_LOCAL].reshape(-1)
        )
        in_maps.append({"tok": tok, **shared})
    res = run_bass_kernel_spmd(
        nc, in_maps, list(range(N_CORES)), trace=TRACE
    )
    out = np.empty((2, B, H), np.float32)
    for c in range(N_CORES):
        o = np.asarray(res.results[c]["out"]).astype(np.float32)
        out[0, c * B_LOCAL:(c + 1) * B_LOCAL, :] = o[0].T
        out[1, c * B_LOCAL:(c + 1) * B_LOCAL, :] = o[1].T
    if TRACE:
        _CACHE["last_exec_time_ns"] = res.exec_time_ns
    return out
